# revision 1
# baseline (speedup 1.0000x reference)
# Neural CDE kernel for 8 Trainium2 NeuronCores — v3.
# Data-parallel over batch: 4096 -> 512/core; 255-step RK4 scan local per core.
#
# v3: full-width [H, 512] stages (no subtile split) to halve instruction and
# weight-load counts. The c-einsum: tanh pieces [H, 2, 512] (bf16) * xd
# (DVE), first tree level on Pool, final 4-way sum on PE via identity-matmul
# PSUM accumulation. RK4 y-combines fold into mm1 PSUM groups with scaled W1
# copies. softplus = Abs, Exp (ACT) + fused DVE relu(x)+poly3(log1p(e));
# layer-1 bias rides in u = W1 z + b1; other biases are rank-1 matmuls.
import sys
sys.path.insert(0, '/opt/trn_rl_repo')

import numpy as np

B_FULL, N_INT, C, H = 4096, 255, 8, 128
N_CORES = 8
B_CORE = B_FULL // N_CORES          # 512
BC = B_CORE
N_STEPS_DEFAULT = 255

# log1p(t) ~ c1 t + c2 t^2 + c3 t^3 on (0,1], max abs err 5.4e-4
SC1, SC2, SC3 = 0.9874542, -0.40841109, 0.11463897

_REGISTERED = {}


def _register_ops():
    if _REGISTERED:
        return _REGISTERED
    import concourse.dve_ops as dve_ops
    from concourse.dve_spec import (Spec, Src0, Src1, C0, C1, C2, Zero,
                                    maxx, lower, _has_src1)
    from concourse.dve_uop import DveOpSpec

    def reg(name, spec):
        if name in dve_ops._SUB_OPCODE_FOR_NAME:
            return next(o for o in dve_ops.OPS if o.name == name)
        shas = {}
        for ver in ("v3", "v4"):
            s = DveOpSpec(name=name, opcode=0, uops=lower(spec, ver=ver),
                          rd1_en=_has_src1(spec))
            shas[ver] = s.sha(ver)
        op = dve_ops.DveOp(name, spec, False, uops_sha=shas)
        dve_ops.OPS.append(op)
        dve_ops.CUSTOM_DVE_SPECS[name] = spec
        dve_ops._SUB_OPCODE_FOR_NAME[name] = max(
            dve_ops._SUB_OPCODE_FOR_NAME.values()) + 1
        return op

    def _sptail_ref(in0, in1, s0, s1, imm2):
        x = in0.astype(np.float32)
        t = in1.astype(np.float32)
        return np.maximum(x, 0) + ((imm2 * t + s1) * t + s0) * t

    _REGISTERED['SPTAIL3'] = reg(
        "SPTAIL3_ANT",
        Spec(body=maxx(Src0, Zero) + (((Src1 * C2 + C1) * Src1) + C0) * Src1,
             reference=_sptail_ref))
    return _REGISTERED


_NC_CACHE = {}


def build(n_steps=N_STEPS_DEFAULT):
    if n_steps in _NC_CACHE:
        return _NC_CACHE[n_steps]
    import concourse.mybir as mybir
    import concourse.tile as tile
    from concourse import bacc

    ops = _register_ops()
    SPTAIL3 = ops['SPTAIL3']

    f32 = mybir.dt.float32
    f32r = mybir.dt.float32r
    bf16 = mybir.dt.bfloat16
    AF = mybir.ActivationFunctionType
    OP = mybir.AluOpType
    OPm, OPa = OP.mult, OP.add

    nc = bacc.Bacc()
    coeffs_d = nc.dram_tensor("coeffs", [BC, N_INT, 32], f32,
                              kind="ExternalInput")
    wd = {}
    for nm in ["W1", "W1_3", "W1_m3", "W1_m", "W1_8", "I128", "W2"]:
        wd[nm] = nc.dram_tensor(nm, [H, H], f32, kind="ExternalInput")
    W3s_d = nc.dram_tensor("W3s", [H, H * C], f32, kind="ExternalInput")
    b3c_d = nc.dram_tensor("b3c", [H, C], f32, kind="ExternalInput")
    b1r_d = nc.dram_tensor("b1r", [1, H], f32, kind="ExternalInput")
    b2r_d = nc.dram_tensor("b2r", [1, H], f32, kind="ExternalInput")
    Wi_d = nc.dram_tensor("Wi", [C, H], f32, kind="ExternalInput")
    bi_d = nc.dram_tensor("bic", [H, 1], f32, kind="ExternalInput")
    Wo_d = nc.dram_tensor("Wo", [H, 1], f32, kind="ExternalInput")
    bo_d = nc.dram_tensor("bo", [1, 1], f32, kind="ExternalInput")
    ones_d = nc.dram_tensor("ones", [1, BC], f32, kind="ExternalInput")
    out_d = nc.dram_tensor("out", [BC, 1], f32, kind="ExternalOutput")

    with tile.TileContext(nc) as tc:
        with tc.tile_pool(name="const", bufs=1) as cp, \
             tc.tile_pool(name="zpool", bufs=2) as zp, \
             tc.tile_pool(name="work", bufs=2) as wp, \
             tc.tile_pool(name="tanh", bufs=6) as thp, \
             tc.tile_pool(name="mul", bufs=6) as mp, \
             tc.tile_pool(name="kpool", bufs=2) as kp, \
             tc.tile_pool(name="xdrep", bufs=5) as xp, \
             tc.tile_pool(name="xdram", bufs=4, space="DRAM") as dp, \
             tc.tile_pool(name="psA", bufs=2, space="PSUM") as psA, \
             tc.tile_pool(name="psK", bufs=1, space="PSUM") as psK, \
             tc.tile_pool(name="ps3", bufs=5, space="PSUM") as ps3:

            # ---- constants ----
            W1_t = cp.tile([H, H], f32r, tag="w1")
            W1_3_t = cp.tile([H, H], f32r, tag="w13")
            W1_m3_t = cp.tile([H, H], f32r, tag="w1m3")
            W1_m_t = cp.tile([H, H], f32r, tag="w1m")
            W1_8_t = cp.tile([H, H], f32r, tag="w18")
            I_t = cp.tile([H, H], f32r, tag="ieye")
            Ib_t = cp.tile([H, H], bf16, tag="ibf")
            W2_t = cp.tile([H, H], f32r, tag="w2")
            W3_t = cp.tile([H, H * C], f32r, tag="w3")
            b3c_t = cp.tile([H, C], f32, tag="b3c")
            W1bf_t = cp.tile([H, H], bf16, tag="w1bf")
            W13bf_t = cp.tile([H, H], bf16, tag="w13bf")
            W18bf_t = cp.tile([H, H], bf16, tag="w18bf")
            b1r_t = cp.tile([1, H], f32r, tag="b1r")
            b2r_t = cp.tile([1, H], f32r, tag="b2r")
            Wi_t = cp.tile([C, H], f32r, tag="wi")
            Wo_t = cp.tile([H, 1], f32r, tag="wo")
            bi_t = cp.tile([H, 1], f32, tag="bi")
            bo_t = cp.tile([1, 1], f32, tag="bo")
            ones_t = cp.tile([1, BC], f32r, tag="ones")
            for t_, d_ in [(W1_t, wd["W1"]), (W1_3_t, wd["W1_3"]),
                           (W1_m3_t, wd["W1_m3"]), (W1_m_t, wd["W1_m"]),
                           (W1_8_t, wd["W1_8"]), (I_t, wd["I128"]),
                           (Ib_t, wd["I128"]), (W2_t, wd["W2"]),
                           (W3_t, W3s_d),
                           (W1bf_t, wd["W1"]), (W13bf_t, wd["W1_3"]),
                           (W18bf_t, wd["W1_8"]),
                           (b1r_t, b1r_d), (b2r_t, b2r_d), (Wi_t, Wi_d),
                           (Wo_t, Wo_d)]:
                nc.gpsimd.dma_start(t_[:], d_[:])
            for t_, d_ in [(bi_t, bi_d), (bo_t, bo_d), (b3c_t, b3c_d)]:
                nc.sync.dma_start(t_[:], d_[:])
            nc.gpsimd.dma_start(ones_t[:], ones_d[:])

            # column order j = p*4 + bh
            coeffs_r = coeffs_d[:].rearrange("(p bh) t (g c) -> p bh t g c",
                                             p=H, g=4)

            # ---- z0 = a[:,0] @ Wi + bi  (feature-major [H, B]) ----
            a0_t = cp.tile([C, BC], f32r, tag="a0")
            nc.gpsimd.dma_start(
                a0_t[:], coeffs_d[:, 0, 0:C].rearrange("(p bh) c -> c (p bh)",
                                                       p=H))
            zps = psA.tile([H, BC], f32, tag="pA")
            nc.tensor.matmul(zps[:], Wi_t[:], a0_t[:], start=True, stop=True)
            zT = zp.tile([H, BC], f32r, tag="z")
            nc.scalar.activation(zT[:], zps[:], AF.Identity, bias=bi_t[:])

            # ---- xd staging ----
            def stage_bcast(t2):
                xdr = dp.tile([1, C * BC], bf16, tag="xdram")
                dst = xdr[0, :].rearrange("(s p b) -> p s b", s=C, p=H)
                nc.sync.dma_start(dst, t2[:])
                rep = xp.tile([H, C, BC], bf16, tag="xdrep")
                nc.sync.dma_start(
                    rep[:].rearrange("h s b -> h (s b)"),
                    xdr[0:1, :].to_broadcast((H, C * BC)))
                return rep

            def xd_make(cf, frac):
                t2 = wp.tile([H, C, 4], bf16, tag="xt2")
                t2w = t2[:].rearrange("p s b -> p b s")
                if frac == 0.0:
                    nc.vector.tensor_scalar_add(t2w, cf[:, :, 1, :], 0.0)
                else:
                    t1 = wp.tile([H, 4, C], f32, tag="xt1")
                    nc.vector.scalar_tensor_tensor(
                        t1[:], cf[:, :, 3, :], float(frac), cf[:, :, 2, :],
                        OPm, OPa)
                    nc.vector.scalar_tensor_tensor(
                        t2w, t1[:], float(frac), cf[:, :, 1, :], OPm, OPa)
                return stage_bcast(t2)

            def load_cf(step):
                cf = wp.tile([H, 4, 4, C], f32, tag="cf")
                nc.sync.dma_start(cf[:], coeffs_r[:, :, step, :, :])
                return cf

            cf0 = load_cf(0)
            xd_prev = xd_make(cf0, 0.0)

            # ---- u0 = W1 z0 + b1 ; p1 seeded for step-0 k1 ----
            p1_t = psA.tile([H, BC], f32, tag="pA", name="p1_init")
            nc.tensor.matmul(p1_t[:], b1r_t[:], ones_t[:],
                             start=True, stop=False)
            nc.tensor.matmul(p1_t[:], W1_t[:], zT[:], start=False, stop=True)
            u_t = wp.tile([H, BC], f32r, tag="u", name="u_init")
            nc.scalar.activation(u_t[:], p1_t[:], AF.Copy)

            def tail(p1, xd_rep, j, feed=None):
                """softplus -> mm2 -> softplus -> mm3 (4 pieces of 2 slots)
                -> tanh -> mult (DVE) -> pair-add (Pool) -> PE sum -> k."""
                a1 = wp.tile([H, BC], f32, tag="spa")
                nc.scalar.activation(a1[:], p1[:], AF.Abs)
                e1 = wp.tile([H, BC], f32, tag="spe")
                nc.scalar.activation(e1[:], a1[:], AF.Exp, scale=-1.0)
                h1 = wp.tile([H, BC], f32r, tag="spha")
                nc.vector._custom_dve(SPTAIL3, out=h1[:], in0=p1[:],
                                      in1=e1[:], s0=SC1, s1=SC2, imm2=SC3)
                p2 = psA.tile([H, BC], f32, tag="pA")
                nc.tensor.matmul(p2[:], b2r_t[:], ones_t[:],
                                 start=True, stop=False)
                nc.tensor.matmul(p2[:], W2_t[:], h1[:], start=False, stop=True)
                a2 = wp.tile([H, BC], f32, tag="spa")
                nc.scalar.activation(a2[:], p2[:], AF.Abs)
                e2 = wp.tile([H, BC], f32, tag="spe")
                nc.scalar.activation(e2[:], a2[:], AF.Exp, scale=-1.0)
                h2 = wp.tile([H, BC], f32r, tag="sphb")
                nc.vector._custom_dve(SPTAIL3, out=h2[:], in0=p2[:],
                                      in1=e2[:], s0=SC1, s1=SC2, imm2=SC3)
                # mm3: 8 single-slot pieces; b3 folded into tanh bias
                p3s = []
                for slot in range(C):
                    p3 = ps3.tile([H, BC], f32, tag="p3")
                    nc.tensor.matmul(p3[:],
                                     W3_t[:, slot * H:(slot + 1) * H], h2[:],
                                     start=True, stop=True)
                    p3s.append(p3)
                Ts = []
                for slot in range(C):
                    T_t = thp.tile([H, BC], bf16, tag="T")
                    nc.scalar.activation(T_t[:], p3s[slot][:], AF.Tanh,
                                         bias=b3c_t[:, slot:slot + 1])
                    Ts.append(T_t)
                pK = psK.tile([H, BC], f32, tag="pK")
                for slot in range(C):
                    P_t = mp.tile([H, BC], bf16, tag="P")
                    nc.vector.tensor_tensor(P_t[:], Ts[slot][:],
                                            xd_rep[:, slot, :], OPm)
                    nc.tensor.matmul(pK[:], Ib_t[:], P_t[:],
                                     start=(slot == 0), stop=(slot == C - 1))
                    if feed is not None:
                        Wf, pn = feed
                        nc.tensor.matmul(pn[:], Wf[:], P_t[:],
                                         start=False, stop=(slot == C - 1))
                k_t = kp.tile([H, BC], f32r, tag=f"k{j}")
                nc.vector.tensor_scalar_add(k_t[:], pK[:], 0.0)
                return k_t

            for step in range(n_steps):
                cf = load_cf(step) if step > 0 else cf0
                xd13 = xd_make(cf, 1.0 / 3.0)
                xd23 = xd_make(cf, 2.0 / 3.0)
                xd1 = xd_make(cf, 1.0)

                # k2's p1 group opens first: I u, then dirP W1/3 P(k1)
                pn2 = psA.tile([H, BC], f32, tag="pA", name="pn2")
                nc.tensor.matmul(pn2[:], I_t[:], u_t[:],
                                 start=True, stop=False)
                k1 = tail(p1_t, xd_prev, 1, feed=(W13bf_t, pn2))

                pn3 = psA.tile([H, BC], f32, tag="pA", name="pn3")
                nc.tensor.matmul(pn3[:], I_t[:], u_t[:],
                                 start=True, stop=False)
                nc.tensor.matmul(pn3[:], W1_m3_t[:], k1[:],
                                 start=False, stop=False)
                k2 = tail(pn2, xd13, 2, feed=(W1bf_t, pn3))

                w1 = wp.tile([H, BC], f32, tag="w1t")
                nc.vector.scalar_tensor_tensor(w1[:], k2[:], 3.0, k1[:],
                                               OPm, OPa)

                pn4 = psA.tile([H, BC], f32, tag="pA", name="pn4")
                nc.tensor.matmul(pn4[:], I_t[:], u_t[:],
                                 start=True, stop=False)
                nc.tensor.matmul(pn4[:], W1_t[:], k1[:],
                                 start=False, stop=False)
                nc.tensor.matmul(pn4[:], W1_m_t[:], k2[:],
                                 start=False, stop=False)
                k3 = tail(pn3, xd23, 3, feed=(W1bf_t, pn4))

                w2 = wp.tile([H, BC], f32r, tag="w2t")
                nc.vector.scalar_tensor_tensor(w2[:], k3[:], 3.0, w1[:],
                                               OPm, OPa)

                last = step == n_steps - 1
                if not last:
                    pn1 = psA.tile([H, BC], f32, tag="pA", name="pn1")
                    nc.tensor.matmul(pn1[:], I_t[:], u_t[:],
                                     start=True, stop=False)
                    nc.tensor.matmul(pn1[:], W1_8_t[:], w2[:],
                                     start=False, stop=False)
                    k4 = tail(pn4, xd1, 4, feed=(W18bf_t, pn1))
                else:
                    k4 = tail(pn4, xd1, 4)

                # z' = (z + w2/8) + k4/8
                zn = zp.tile([H, BC], f32r, tag="z")
                w3_ = wp.tile([H, BC], f32r, tag="w3t")
                nc.vector.scalar_tensor_tensor(w3_[:], w2[:], 0.125, zT[:],
                                               OPm, OPa)
                nc.vector.scalar_tensor_tensor(zn[:], k4[:], 0.125, w3_[:],
                                               OPm, OPa)
                if not last:
                    p1_t = pn1
                    u_t = wp.tile([H, BC], f32r, tag="u")
                    nc.scalar.activation(u_t[:], pn1[:], AF.Copy)
                zT = zn
                xd_prev = xd1

            # ---- out = zT @ W_out + b_out ----
            ops_ = psK.tile([H, BC], f32, tag="pK")
            nc.tensor.matmul(ops_[0:1, :], Wo_t[:], zT[:],
                             start=True, stop=True)
            ot = cp.tile([1, BC], f32, tag="outs")
            nc.scalar.activation(ot[:], ops_[0:1, :], AF.Identity,
                                 bias=bo_t[:])
            nc.sync.dma_start(
                out_d[:].rearrange("(p bh) one -> one (p bh)", p=H), ot[:])

    nc.finalize()
    _NC_CACHE[n_steps] = nc
    return nc


def host_inputs(inputs, core):
    coeffs = np.ascontiguousarray(
        inputs["coeffs"][core * BC:(core + 1) * BC]).astype(np.float32)
    W1 = inputs["W1"].astype(np.float32)
    W3 = inputs["W3"].astype(np.float32)
    # W3s[h_in, slot*H + h_out] = W3[h_in, h_out*C + slot]
    W3s = np.ascontiguousarray(
        W3.reshape(H, H, C).transpose(0, 2, 1).reshape(H, H * C))
    b3c = np.ascontiguousarray(
        inputs["b3"].astype(np.float32).reshape(H, C))
    return dict(
        coeffs=coeffs,
        W1=W1, W1_3=W1 / 3.0, W1_m3=-W1 / 3.0, W1_m=-W1, W1_8=W1 / 8.0,
        I128=np.eye(H, dtype=np.float32),
        W2=inputs["W2"].astype(np.float32),
        W3s=W3s, b3c=b3c,
        b1r=inputs["b1"].reshape(1, H).astype(np.float32),
        b2r=inputs["b2"].reshape(1, H).astype(np.float32),
        Wi=np.ascontiguousarray(inputs["W_init"].astype(np.float32)),
        bic=inputs["b_init"].reshape(H, 1).astype(np.float32),
        Wo=inputs["W_out"].reshape(H, 1).astype(np.float32),
        bo=inputs["b_out"].reshape(1, 1).astype(np.float32),
        ones=np.ones((1, BC), np.float32),
    )


def kernel(**inputs):
    return _run(N_STEPS_DEFAULT, False, inputs)


def _run(n_steps, trace, inputs):
    from concourse.bass_utils import run_bass_kernel_spmd
    nc = build(n_steps)
    in_maps = [host_inputs(inputs, i) for i in range(N_CORES)]
    res = run_bass_kernel_spmd(nc, in_maps, core_ids=list(range(N_CORES)),
                               trace=trace)
    out = np.concatenate([res.results[i]["out"] for i in range(N_CORES)],
                         axis=0)
    _run.last_result = res
    return out

